# revision 5
# baseline (speedup 1.0000x reference)
"""MultiHeadSelfAttention (qk-LayerNorm) on 8 TRN2 NeuronCores.

Problem (B=4, N=2048, C=1024, H=16, D=64, fp32):
    qkv = x @ W_qkv + b_qkv ; q,k,v = split(qkv)
    q = LN(q)*scale ; k = LN(k)          (LN over full C)
    attn = softmax(q_h @ k_h^T) per head ; o = attn @ v_h
    out = concat_heads(o) @ W_proj + b_proj

Sharding: core i handles batch b=i//2 and query-half i%2 (1024 query rows).
K/V for the full sequence are computed on both half-cores (no collectives).

Design: all matmul operands fp16 -> x^T, K^T, Q^T, O^T and all weights stay
SBUF-resident (no DRAM staging round trip). The V GEMM runs inside the
attention phase as PE filler work so the tensor engine never idles long
enough for the HAM clock gate to re-throttle it to 1.2 GHz (an earlier
fp32 version lost 666us of 919us to K=4/8 throttling that way). Attention
runs query-split (512-query halves) with both heads of a channel-pair
interleaved per key tile: PE stream per iter = 4 V-fill MMs + QK_A +
PV_A(prev) + QK_B + PV_B(prev) (full-row PVs between the 64-row QKs keep
the stream dense), one [128,1024] exp on ACT (A|B halves), PSUM = 2x S(2)
+ po_A(1) + po_B(1) + vfill(2) = 8 banks. Softmax denominators ride an
appended ones column in V (M=65 PV); epilogue = PSUM evacuation copy +
reciprocal_approx_fast + partition_broadcast + multiply, which frees the
accumulator bank in ~0.6us. P1 computes K then Q token-major (LN stats on
fp32 PSUM), PE-transposes to channel-major with gamma/beta (and softmax
scale for q) folded into the evacuation; P3 projects from resident O^T.
"""
import numpy as np
from contextlib import ExitStack

import concourse.bass as bass
from concourse import bacc
import concourse.tile as tile
import concourse.mybir as mybir
from concourse.masks import make_identity

dt = mybir.dt
AF = mybir.ActivationFunctionType
OP = mybir.AluOpType
ts = bass.ts

B, N, C = 4, 2048, 1024
H, D = 16, 64
NQ = 1024            # query rows per core
SCALE = D ** -0.5
EPS = 1e-6
TT = N // 128        # 16 token tiles (full seq)
TQ = NQ // 128       # 8 query tiles
CT = C // 128        # 8 channel tiles (= head pairs)
BF = dt.float16


def build_nc(reps=1, with_bias=False):
    nc = bacc.Bacc()
    xT = nc.dram_tensor("xT", [128, CT, N], BF, kind="ExternalInput")
    wk_d = nc.dram_tensor("wk_d", [128, CT, C], BF, kind="ExternalInput")
    wq_d = nc.dram_tensor("wq_d", [128, CT, C], BF, kind="ExternalInput")
    wv_d = nc.dram_tensor("wv_d", [128, CT, C], BF, kind="ExternalInput")
    wp_d = nc.dram_tensor("wp_d", [128, CT, C], BF, kind="ExternalInput")
    gq = nc.dram_tensor("gq", [C], dt.float32, kind="ExternalInput")
    bq = nc.dram_tensor("bq", [C], dt.float32, kind="ExternalInput")
    gk = nc.dram_tensor("gk", [C], dt.float32, kind="ExternalInput")
    bk = nc.dram_tensor("bk", [C], dt.float32, kind="ExternalInput")
    bqkv = nc.dram_tensor("bqkv", [3 * C], BF, kind="ExternalInput")
    bproj = nc.dram_tensor("bproj", [C], BF, kind="ExternalInput")
    ones128 = nc.dram_tensor("ones128", [128], BF, kind="ExternalInput")
    onesvp = nc.dram_tensor("onesvp", [128, TT], BF, kind="ExternalInput")
    epsv = nc.dram_tensor("epsv", [128], dt.float32, kind="ExternalInput")
    out = nc.dram_tensor("out", [NQ, C], dt.float32, kind="ExternalOutput")

    with tile.TileContext(nc) as tc, ExitStack() as top:
        const = top.enter_context(tc.tile_pool(name="const", bufs=1))
        res = top.enter_context(tc.tile_pool(name="res", bufs=1))
        wg_p = top.enter_context(tc.tile_pool(name="wg", bufs=2))

        # ---- residents ----
        xT_sb = res.tile([128, CT, N], BF)    # x^T, channel-major
        knT = res.tile([128, CT, N], BF)      # LN'd K^T
        qnT = res.tile([128, CT, NQ], BF)     # LN'd+scaled Q^T
        oT = res.tile([128, CT, NQ], BF)      # attention out (normalized)
        wv_sb = res.tile([128, CT, C], BF)    # W_v resident through P2

        # critical-path DMA first: wk (split across queues) + first x^T chunk
        wk_t = wg_p.tile([128, CT, C], BF, tag="wg", name="wk_t")
        for kk in range(4):
            nc.sync.dma_start(wk_t[:, 2 * kk:2 * kk + 2, :],
                              wk_d[:, 2 * kk:2 * kk + 2, :])
        nc.sync.dma_start(xT_sb[:, :, 0:512], xT[:, :, 0:512])

        # ---- constants ----
        ident = const.tile([128, 128], BF)
        make_identity(nc, ident[:])
        ones1 = const.tile([1, 128], BF)
        nc.sync.dma_start(ones1[:], ones128.rearrange("(o n) -> o n", o=1))
        eps_t = const.tile([128, 1], dt.float32)
        nc.sync.dma_start(eps_t[:], epsv.rearrange("(p o) -> p o", o=1))
        gq_t = const.tile([128, CT], dt.float32)
        bq_t = const.tile([128, CT], dt.float32)
        gk_t = const.tile([128, CT], dt.float32)
        bk_t = const.tile([128, CT], dt.float32)
        for t_, d_ in ((gq_t, gq), (bq_t, bq), (gk_t, gk), (bk_t, bk)):
            nc.sync.dma_start(t_[:], d_.rearrange("(ct p) -> p ct", p=128))
        bqkv_t = const.tile([1, 3 * C], BF)
        nc.sync.dma_start(bqkv_t[:], bqkv.rearrange("(o n) -> o n", o=1))
        bproj_t = const.tile([1, C], BF)
        nc.sync.dma_start(bproj_t[:], bproj.rearrange("(o n) -> o n", o=1))
        vp_p = top.enter_context(tc.tile_pool(name="vp", bufs=2))
        vp_tiles = {}

        def vp_alloc(pair):
            """Allocate a V tile; softmax ones columns arrive by DMA so no
            engine pass sits on the P2-entry critical path."""
            vp = vp_p.tile([128, TT, 130], BF, tag="vp", name=f"vp{pair}")
            nc.sync.dma_start(vp[:, :, 64], onesvp[:, :])
            nc.sync.dma_start(vp[:, :, 129], onesvp[:, :])
            vp_tiles[pair] = vp

        vp_alloc(0)

        # rest of x^T, then wq/wv prefetch
        for chk in range(1, 4):
            nc.sync.dma_start(xT_sb[:, :, ts(chk, 512)],
                              xT[:, :, ts(chk, 512)])
        wq_t = wg_p.tile([128, CT, C], BF, tag="wg", name="wq_t")
        nc.sync.dma_start(wq_t[:], wq_d[:, :, :])
        nc.sync.dma_start(wv_sb[:], wv_d[:, :, :])

        for _rep in range(reps):
            # ============ P1: K then Q (GEMM + LN + transpose) ============
            with ExitStack() as p1:
                ln_p = p1.enter_context(tc.tile_pool(name="ln", bufs=3))
                tk_p = p1.enter_context(tc.tile_pool(name="tk", bufs=2))
                ps_p = p1.enter_context(tc.tile_pool(name="ps1", bufs=3, space="PSUM"))
                pst_p = p1.enter_context(tc.tile_pool(name="pst", bufs=2, space="PSUM"))

                def qkv_psum(ps, w_t, oc_base, tok0):
                    """token-major x@W chunk for 128 tokens starting at tok0"""
                    for kt in range(CT):
                        for ch in range(2):
                            nc.tensor.matmul(
                                ps[:, ts(ch, 512)],
                                xT_sb[:, kt, tok0:tok0 + 128],
                                w_t[:, kt, ts(ch, 512)],
                                start=(kt == 0),
                                stop=(not with_bias and kt == CT - 1),
                                skip_group_check=True)
                    if with_bias:
                        for ch in range(2):
                            nc.tensor.matmul(
                                ps[:, ts(ch, 512)], ones1[:],
                                bqkv_t[:, oc_base + ch * 512:oc_base + (ch + 1) * 512],
                                start=False, stop=True, skip_group_check=True)

                def ln_stats(ps_tok):
                    sum_t = ln_p.tile([128, 1], dt.float32, tag="sum")
                    nc.vector.tensor_reduce(sum_t[:], ps_tok[:], mybir.AxisListType.X, OP.add)
                    neg_mu = ln_p.tile([128, 1], dt.float32, tag="nmu")
                    nc.vector.tensor_scalar_mul(neg_mu[:], sum_t[:], -1.0 / C)
                    sq = ln_p.tile([128, C], dt.float32, tag="sq")
                    ssq = ln_p.tile([128, 1], dt.float32, tag="ssq")
                    nc.scalar.activation(sq[:], ps_tok[:], AF.Square, accum_out=ssq[:])
                    msq = ln_p.tile([128, 1], dt.float32, tag="msq")
                    nc.vector.tensor_tensor(msq[:], neg_mu[:], neg_mu[:], op=OP.mult)
                    var = ln_p.tile([128, 1], dt.float32, tag="var")
                    nc.vector.tensor_scalar(var[:], ssq[:], 1.0 / C, msq[:],
                                            op0=OP.mult, op1=OP.subtract)
                    sv = ln_p.tile([128, 1], dt.float32, tag="sv")
                    nc.scalar.activation(sv[:], var[:], AF.Sqrt, bias=eps_t[:])
                    rstd = ln_p.tile([128, 1], dt.float32, tag="rstd")
                    with nc.allow_low_precision(reason="layernorm rstd"):
                        nc.vector.reciprocal(rstd[:], sv[:])
                    return neg_mu, rstd

                def ln_transpose(ps_tok, g_t, b_t, dst, dst_off):
                    """LN-apply -> bf16 tok -> PE transpose -> gamma/beta evac
                    into dst[:, ct, dst_off:dst_off+128]."""
                    neg_mu, rstd = ln_stats(ps_tok)
                    tok = tk_p.tile([128, C], BF, tag="tok")
                    nc.vector.tensor_scalar(tok[:], ps_tok[:], neg_mu[:], rstd[:],
                                            op0=OP.add, op1=OP.mult)
                    for ct in range(CT):
                        ps_t = pst_p.tile([128, 128], BF, tag="ps_t")
                        nc.tensor.matmul(ps_t[:], tok[:, ts(ct, 128)], ident[:],
                                         is_transpose=True, start=True, stop=True,
                                         skip_group_check=True)
                        nc.vector.tensor_scalar(
                            dst[:, ct, dst_off:dst_off + 128], ps_t[:],
                            g_t[:, ct:ct + 1], b_t[:, ct:ct + 1],
                            op0=OP.mult, op1=OP.add)

                # K over full sequence; transpose lags one tile
                pend = None
                for tt in range(TT):
                    ps_k = ps_p.tile([128, C], dt.float32, tag="ps_k")
                    qkv_psum(ps_k, wk_t, C, tt * 128)
                    if pend is not None:
                        ln_transpose(*pend)
                    pend = (ps_k, gk_t, bk_t, knT, tt * 128)
                ln_transpose(*pend)

                # Q over this core's query half
                pend = None
                for tq in range(TQ):
                    ps_q = ps_p.tile([128, C], dt.float32, tag="ps_k")
                    qkv_psum(ps_q, wq_t, 0, tq * 128)
                    if pend is not None:
                        ln_transpose(*pend)
                    pend = (ps_q, gq_t, bq_t, qnT, tq * 128)
                ln_transpose(*pend)

            # ============ P2: attention with fused V GEMM ============
            with ExitStack() as p2:
                pt_p = p2.enter_context(tc.tile_pool(name="pt", bufs=3))
                nz_p = p2.enter_context(tc.tile_pool(name="nz", bufs=2))
                ps_s = p2.enter_context(tc.tile_pool(name="ps_s", bufs=2, space="PSUM"))
                ps_oa = p2.enter_context(tc.tile_pool(name="ps_oa", bufs=1, space="PSUM"))
                ps_ob = p2.enter_context(tc.tile_pool(name="ps_ob", bufs=1, space="PSUM"))
                ps_v = p2.enter_context(tc.tile_pool(name="ps_v", bufs=2, space="PSUM"))

                def vfill_steps(pair):
                    """Generator of V-fill closures for `pair`: 32 steps of
                    4 MMs (half a token tile each), plus evacuations."""
                    vp = vp_tiles[pair]
                    oc = pair * 128
                    for tt in range(TT):
                        psv = ps_v.tile([128, 128], dt.float32, tag="psv")

                        def h1(psv=psv, tt=tt):
                            for kt in range(4):
                                nc.tensor.matmul(
                                    psv[:], xT_sb[:, kt, ts(tt, 128)],
                                    wv_sb[:, kt, oc:oc + 128],
                                    start=(kt == 0), stop=False,
                                    skip_group_check=True)
                        yield h1

                        def h2(psv=psv, tt=tt, vp=vp):
                            for kt in range(4, CT):
                                nc.tensor.matmul(
                                    psv[:], xT_sb[:, kt, ts(tt, 128)],
                                    wv_sb[:, kt, oc:oc + 128],
                                    start=False,
                                    stop=(not with_bias and kt == CT - 1),
                                    skip_group_check=True)
                            if with_bias:
                                nc.tensor.matmul(
                                    psv[:], ones1[:],
                                    bqkv_t[:, 2 * C + oc:2 * C + oc + 128],
                                    start=False, stop=True,
                                    skip_group_check=True)
                            nc.vector.tensor_copy(vp[:, tt, 0:64], psv[:, 0:64])
                            nc.vector.tensor_copy(vp[:, tt, 65:129], psv[:, 64:128])
                        yield h2

                def flash(pair, qh, vfill):
                    """Flash attention for 512 queries (qh half), both heads,
                    V-fill steps for a later pair interleaved."""
                    vp = vp_tiles[pair]
                    poa = ps_oa.tile([65, 512], dt.float32, tag="poa")
                    pob = ps_ob.tile([65, 512], dt.float32, tag="pob")
                    q0 = qh * 512

                    def qk(kt, b0, s0):
                        nc.tensor.matmul(
                            pss_cur[:, s0:s0 + 512],
                            knT[b0:b0 + 64, pair, ts(kt, 128)],
                            qnT[b0:b0 + 64, pair, q0:q0 + 512],
                            start=True, stop=True, skip_group_check=True)

                    def pv(kt, pT, po, v0, s0):
                        nc.tensor.matmul(
                            po[:], vp[:, kt, v0:v0 + 65], pT[:, s0:s0 + 512],
                            start=(kt == 0), stop=(kt == TT - 1),
                            skip_group_check=True)

                    # PE order per iter: vfill, QK_A(k), PV_A(k-1), QK_B(k),
                    # PV_B(k-1) — the full-row PVs between the two 64-row QKs
                    # keep the PE stream dense (QK pair would pack otherwise
                    # and leave the PE idler than ACT's exp).
                    prev = None
                    for kt in range(TT):
                        if vfill is not None:
                            step = next(vfill, None)
                            if step is not None:
                                step()
                        pss_cur = ps_s.tile([128, 1024], dt.float32, tag="pss")
                        qk(kt, 0, 0)
                        if prev is not None:
                            pv(kt - 1, prev, poa, 0, 0)
                        qk(kt, 64, 512)
                        if prev is not None:
                            pv(kt - 1, prev, pob, 65, 512)
                        pT = pt_p.tile([128, 1024], BF, tag="pT")
                        nc.scalar.activation(pT[:], pss_cur[:], AF.Exp)
                        prev = pT
                    pv(TT - 1, prev, poa, 0, 0)
                    pv(TT - 1, prev, pob, 65, 512)

                    for po, b0 in ((poa, 0), (pob, 64)):
                        # single evacuation frees the PSUM bank immediately
                        # (the next qh's PV(0) reuses it ~1.5us in)
                        ev = nz_p.tile([65, 512], dt.float32, tag="ev")
                        nc.vector.tensor_copy(ev[:], po[:])
                        den = nz_p.tile([1, 512], dt.float32, tag="den")
                        nc.vector.tensor_copy(den[:], ev[64:65, :])
                        rc = nz_p.tile([1, 512], dt.float32, tag="rc")
                        nc.vector.reciprocal_approx_fast(rc[:], den[:])
                        bc = nz_p.tile([64, 512], dt.float32, tag="bc")
                        nc.gpsimd.partition_broadcast(bc[:], rc[0:1, :])
                        nc.vector.tensor_tensor(
                            oT[b0:b0 + 64, pair, q0:q0 + 512],
                            ev[0:64, :], bc[:], op=OP.mult)

                # fill pair 0 densely, then run pairs with lookahead fill
                for step in vfill_steps(0):
                    step()
                wp_t = None
                for pair in range(CT):
                    if pair + 1 < CT:
                        vp_alloc(pair + 1)
                        vf = vfill_steps(pair + 1)
                    else:
                        vf = None
                    flash(pair, 0, vf)
                    flash(pair, 1, vf)
                    if vf is not None:
                        for step in vf:
                            step()  # drain any unconsumed fill steps
                    if pair == 0:
                        # W_proj prefetch mid-P2: issuing it at a phase
                        # boundary stalls the pool-transition barrier on it
                        wp_t = wg_p.tile([128, CT, C], BF, tag="wg",
                                         name="wp_t")
                        nc.sync.dma_start(wp_t[:], wp_d[:, :, :])

            # ============ P3: projection ============
            with ExitStack() as p3:
                os_p = p3.enter_context(tc.tile_pool(name="os", bufs=3))
                ps3_p = p3.enter_context(tc.tile_pool(name="ps3", bufs=4, space="PSUM"))

                for tq in range(TQ):
                    ost = os_p.tile([128, C], dt.float32, tag="ost")
                    pss = [ps3_p.tile([128, 512], dt.float32, tag=f"ps{oc}",
                                      name=f"ps3_{oc}")
                           for oc in range(2)]
                    for ct in range(CT):
                        for oc in range(2):
                            nc.tensor.matmul(
                                pss[oc][:], oT[:, ct, ts(tq, 128)],
                                wp_t[:, ct, ts(oc, 512)],
                                start=(ct == 0),
                                stop=(not with_bias and ct == CT - 1),
                                skip_group_check=True)
                    for oc in range(2):
                        if with_bias:
                            nc.tensor.matmul(
                                pss[oc][:], ones1[:], bproj_t[:, ts(oc, 512)],
                                start=False, stop=True, skip_group_check=True)
                        nc.vector.tensor_copy(ost[:, ts(oc, 512)], pss[oc][:])
                    nc.sync.dma_start(out[ts(tq, 128), :], ost[:])

    nc.compile()
    return nc


_NC = {}


def _get_nc(with_bias=False):
    key = with_bias
    if key not in _NC:
        _NC[key] = build_nc(with_bias=with_bias)
    return _NC[key]


def _shard_inputs(inputs):
    bf16 = np.float16
    x = np.asarray(inputs["x"], dtype=np.float32)
    wqkv = np.asarray(inputs["W_qkv"], dtype=np.float32)
    wproj = np.asarray(inputs["W_proj"], dtype=np.float32)

    def wlayout(w):  # [C, C] -> [128, CT, C], partition = cin % 128
        return np.ascontiguousarray(
            w.reshape(CT, 128, C).transpose(1, 0, 2)).astype(bf16)

    shared = {
        "wq_d": wlayout(wqkv[:, 0:C]),
        "wk_d": wlayout(wqkv[:, C:2 * C]),
        "wv_d": wlayout(wqkv[:, 2 * C:3 * C]),
        "wp_d": wlayout(wproj),
        "bqkv": np.asarray(inputs["b_qkv"], dtype=np.float32).astype(bf16),
        "bproj": np.asarray(inputs["b_proj"], dtype=np.float32).astype(bf16),
        "ones128": np.ones(128, dtype=np.float32).astype(bf16),
        "onesvp": np.ones((128, TT), dtype=np.float32).astype(bf16),
        "epsv": np.full(128, EPS, dtype=np.float32),
        "gq": np.asarray(inputs["q_gamma"], dtype=np.float32) * np.float32(SCALE),
        "bq": np.asarray(inputs["q_beta"], dtype=np.float32) * np.float32(SCALE),
        "gk": np.asarray(inputs["k_gamma"], dtype=np.float32),
        "bk": np.asarray(inputs["k_beta"], dtype=np.float32),
    }
    in_maps = []
    for core in range(8):
        b, half = core // 2, core % 2
        m = dict(shared)
        # xT[p, ct, t] = x[b][t, ct*128+p]; Q group indexes the query half
        xt = np.ascontiguousarray(
            x[b].T.reshape(CT, 128, N).transpose(1, 0, 2)).astype(bf16)
        if half == 1:
            # roll so this core's queries sit at xT[:, :, 0:NQ]
            xt = np.ascontiguousarray(np.roll(xt, -NQ, axis=2))
        m["xT"] = xt
        in_maps.append(m)
    return in_maps


def kernel(**inputs) -> np.ndarray:
    from concourse.bass_utils import run_bass_kernel_spmd
    zero_bias = (not np.any(np.asarray(inputs["b_qkv"]))
                 and not np.any(np.asarray(inputs["b_proj"])))
    nc = _get_nc(with_bias=not zero_bias)
    in_maps = _shard_inputs(inputs)
    res = run_bass_kernel_spmd(nc, in_maps, core_ids=list(range(8)))
    out = np.empty((B, N, C), dtype=np.float32)
    for core in range(8):
        b, half = core // 2, core % 2
        out[b, half * NQ:(half + 1) * NQ, :] = res.results[core]["out"]
    return out
